# revision 25
# baseline (speedup 1.0000x reference)
"""Trainium2 Bass kernel: per-pixel 5x5 kernel application (KPN-style).

    out[b,c,y,x] = sum_{i,j} softmax(kernels[b,:,y,x])[i*5+j]
                   * zpad(data)[b,c,y+i,x+j]          (i,j in 0..4, r=2)

Sharding (8 NeuronCores, pure data parallel, no collectives):
    core = (b, H-half): 4 batches x 2 row-bands of 360 rows.

Band layout: partition p = x-band of 10 columns (128 bands x 10 = 1280).
Rows live in the free dimension, so BOTH the di (row) and dj (col) tap
shifts become free-dim AP offsets -- no shift matrices, no partition
crossing.  The host ships the data pre-expanded over dj (D[band, row,
c, dj, x] = data[c, row, 10*band + x + dj - 2]) so one DVE op per
(di, c) covers all five dj taps with 4-byte-aligned 2x-mode APs.

Per accumulation tile (20/40-row warmup tiles, then 45 rows):
    - ACT: E = exp(logits), in place, one op (fp16).
    - DVE: per (di, c): ONE batched product q = E * D ([5, nr, 10]).
    - PE:  identity-lhsT matmuls accumulate the 75 tap planes and the
      25 exp planes into 4 PSUM banks (start/stop per bank).  The
      stationary operand never changes, so the PE stays warm.
    - epilogue (issued one di-group into the NEXT tile so nothing
      stalls on the PSUM-stop semaphores): R = recip_approx(sumexp)
      (DVE) + fp16 cast; P: PSUM->SBUF fp16 copy on ACT (issued ahead
      of the next exp); out = P * R (DVE 2x); store.

DMA: all loads ride SWDGE (gpsimd) so descriptors spray across all 16
SDMA engines with one contiguous run per partition.  The first kE load
is emitted BEFORE the D chunks so nothing queues ahead of the
exp->product critical path at startup.  Stores ride the HWDGE rings
(engines 0-3, otherwise idle).  GpSimd tensor ops measured 6x slower
than DVE and slow concurrent DVE ops -- not used.

kernel(**inputs) takes the FULL inputs and returns the FULL output.
"""

import numpy as np
from numpy.lib.stride_tricks import sliding_window_view

B, C, H, W, KW = 4, 3, 720, 1280, 5
NCORES = 8
HS = H // 2            # 360 output rows per shard
NB = 128               # x-bands (partitions)
BW = 10                # band width (output columns per partition)
DR = HS + 4            # 364 data rows incl. 2+2 halo rows
TAPS = KW * KW

# accumulation tiles: small warmup tiles shorten the startup pipeline,
# a small final tile shortens the drain
TILES = ([(0, 20), (20, 40)] + [(60 + 45 * k, 45) for k in range(6)]
         + [(330, 30)])
# D chunk rows, chunk i loaded behind tile i's kE load; fine-grained so
# no kE load ever queues behind a multi-MB D transfer on the SWDGE ring
DCHUNKS = [(0, 50), (50, 96), (96, 142), (142, 188),
           (188, 256), (256, 320), (320, 364)]

_CACHE = {}


def _build_program():
    import concourse.bacc as bacc
    import concourse.mybir as mybir
    from concourse.bass import AP
    from concourse import tile

    f32 = mybir.dt.float32
    f16 = mybir.dt.float16

    nc = bacc.Bacc(
        "TRN2",
        target_bir_lowering=False,
        debug=False,
        enable_asserts=False,
        num_devices=NCORES,
    )
    d_ke = nc.dram_tensor("ke", [NB, HS, TAPS * BW], f16, kind="ExternalInput")
    d_dx = nc.dram_tensor("dx", [NB, DR, C * KW * BW], f16, kind="ExternalInput")
    d_out = nc.dram_tensor("out", [NB, HS, C * BW], f16, kind="ExternalOutput")

    d_id = nc.inline_tensor(np.eye(NB, dtype=np.float16), "ident")

    KE_ROW = TAPS * BW          # 250 elems per (band, row)
    D_ROW = C * KW * BW         # 150 elems per (band, row)

    with tile.TileContext(nc) as tc:
        with tc.tile_pool(name="const", bufs=1) as cpool, \
             tc.tile_pool(name="kt", bufs=2) as kpool, \
             tc.tile_pool(name="dt", bufs=1) as dpool, \
             tc.tile_pool(name="qt", bufs=6) as qpool, \
             tc.tile_pool(name="rt", bufs=2) as rpool, \
             tc.tile_pool(name="ot", bufs=2) as opool, \
             tc.tile_pool(name="ps", bufs=2, space="PSUM") as ppool:

            id_sb = cpool.tile([NB, NB], f16)
            nc.sync.dma_start(out=id_sb[:], in_=d_id.ap())

            DX = dpool.tile([NB, DR, C, KW, BW], f16, tag="dx")

            # PE pre-warm: ~3.4us of back-to-back matmuls off the
            # already-resident identity flips the HAM clock gate to
            # 8/8 before the first real accumulation arrives.  The
            # polluted bank is cleared by that group's start=True.
            warm = ppool.tile([NB, 4, 512], f32, tag="pacc")
            for w in range(32):
                nc.tensor.matmul(
                    out=warm[:, 3, 0:NB],
                    lhsT=id_sb[:],
                    rhs=id_sb[:],
                    start=(w == 0),
                    stop=(w == 31),
                )


            st_eng = [nc.sync, nc.scalar]

            # deferred epilogue: normalize+store of tile t runs while
            # tile t+1's products stream, so the DVE never stalls
            # waiting for the PE to drain the accumulation group
            pending = []

            def ep_act():
                # P: PSUM -> SBUF fp16 on ACT.  Issued BEFORE the next
                # exp so it isn't stuck behind a 9.7us ACT op when the
                # DVE-side normalize needs it.
                if not pending:
                    return
                ent = pending[0]
                nr, pacc = ent[2], ent[3]
                Pst = opool.tile([NB, nr, C, BW], f16, tag="pst")
                p_view = AP(
                    pacc[:].tensor, 0,
                    [[4 * 512, NB], [BW, nr], [512, C], [1, BW]],
                )
                nc.scalar.activation(
                    Pst[:], p_view, mybir.ActivationFunctionType.Copy,
                )
                ent[4] = Pst

            def ep_dve():
                # normalize+store, issued one di-group into the next
                # tile so the PSUM-stop semaphores are already clear
                if not pending:
                    return
                t, r0, nr, pacc, Pst = pending.pop(0)
                fd = nr * BW
                R32 = rpool.tile([NB, nr, BW], f32, tag="r32")
                nc.vector.reciprocal_approx_fast(
                    out=R32[:].rearrange("p r x -> p (r x)"),
                    in_=pacc[:, 3, 0:fd],
                )
                R16 = rpool.tile([NB, nr, BW], f16, tag="r16")
                nc.vector.tensor_copy(
                    R16[:].rearrange("p r x -> p (r x)"),
                    R32[:].rearrange("p r x -> p (r x)"),
                )
                outst = opool.tile([NB, nr, C, BW], f16, tag="o")
                r_bc = AP(
                    R16[:].tensor, 0,
                    [[nr * BW, NB], [BW, nr], [0, C], [1, BW]],
                )
                nc.vector.tensor_tensor(
                    outst[:], Pst[:], r_bc, mybir.AluOpType.mult)
                st_eng[t % 2].dma_start(
                    out=d_out.ap()[:, r0:r0 + nr],
                    in_=outst[:].rearrange("p r c x -> p r (c x)"),
                )

            for t, (r0, nr) in enumerate(TILES):
                fd = nr * BW
                kE = kpool.tile([NB, nr, TAPS, BW], f16, tag="ke")
                nc.gpsimd.dma_start(
                    out=kE[:].rearrange("p r t x -> p r (t x)"),
                    in_=d_ke.ap()[:, r0:r0 + nr],
                )
                # D chunks ride the same SWDGE queue, behind the kE
                # loads whose consumers they race
                if t < len(DCHUNKS):
                    ra, rb = DCHUNKS[t]
                    nc.gpsimd.dma_start(
                        out=DX[:, ra:rb].rearrange("p r c k x -> p r (c k x)"),
                        in_=d_dx.ap()[:, ra:rb],
                    )

                ep_act()
                # exp in place: kE holds E from here on
                keflat = kE[:].rearrange("p r t x -> p (r t x)")
                nc.scalar.activation(
                    keflat, keflat, mybir.ActivationFunctionType.Exp,
                )
                eap = kE[:]

                pacc = ppool.tile([NB, 4, 512], f32, tag="pacc")

                for di in range(KW):
                    # products: q = E * D, all 5 dj in one op
                    qs = []
                    for c in range(C):
                        q = qpool.tile([NB, KW, nr, BW], f16, tag="q")
                        e_v = AP(
                            eap.tensor, (KW * di) * BW,
                            [[nr * KE_ROW, NB], [BW, KW], [KE_ROW, nr], [1, BW]],
                        )
                        d_v = AP(
                            DX[:].tensor, (r0 + di) * D_ROW + c * KW * BW,
                            [[DR * D_ROW, NB], [BW, KW], [D_ROW, nr], [1, BW]],
                        )
                        nc.vector.tensor_tensor(
                            q[:], e_v, d_v, mybir.AluOpType.mult)
                        qs.append(q)
                    # sumexp: 5 identity matmuls straight off E (no DVE dep)
                    for k in range(KW):
                        tp = KW * di + k
                        nc.tensor.matmul(
                            out=pacc[:, 3, 0:fd],
                            lhsT=id_sb[:],
                            rhs=eap[:, :, tp, :],
                            start=(tp == 0),
                            stop=(tp == TAPS - 1),
                        )
                    # tap accumulation
                    for c in range(C):
                        for k in range(KW):
                            nc.tensor.matmul(
                                out=pacc[:, c, 0:fd],
                                lhsT=id_sb[:],
                                rhs=qs[c][:, k],
                                start=(di == 0 and k == 0),
                                stop=(di == KW - 1 and k == KW - 1),
                            )
                    if di == 0:
                        ep_dve()

                pending.append([t, r0, nr, pacc, None])

            ep_act()
            ep_dve()

    nc.compile()
    return nc


def get_program():
    if "nc" not in _CACHE:
        _CACHE["nc"] = _build_program()
    return _CACHE["nc"]


def make_shards(data: np.ndarray, kernels: np.ndarray):
    """Full inputs -> per-core input maps (band layout, fp16)."""
    data = np.asarray(data, dtype=np.float32)
    kernels = np.asarray(kernels, dtype=np.float32)

    kf = kernels.astype(np.float16)                   # [B, 25, H, W]
    dpad = np.zeros((B, C, H + 4, W + 6), dtype=np.float16)
    dpad[:, :, 2:H + 2, 2:W + 2] = data

    in_maps = []
    for core in range(NCORES):
        b, hh = divmod(core, 2)
        r0 = hh * HS
        ks = kf[b, :, r0:r0 + HS, :]                  # [25, 360, 1280]
        ke = np.ascontiguousarray(
            ks.reshape(TAPS, HS, NB, BW).transpose(2, 1, 0, 3)
        ).reshape(NB, HS, TAPS * BW)
        dsl = dpad[b, :, r0:r0 + DR, :]               # [3, 364, 1286]
        win = sliding_window_view(dsl, BW, axis=2)    # [3, 364, 1277, 10]
        # dx[p, row, c, dj, x] = data[c, row, 10p + x + dj - 2]
        dx = np.empty((NB, DR, C, KW, BW), dtype=np.float16)
        for dj in range(KW):
            dx[:, :, :, dj, :] = (
                win[:, :, dj:dj + NB * BW:BW].transpose(2, 1, 0, 3))
        in_maps.append({
            "ke": ke,
            "dx": dx.reshape(NB, DR, C * KW * BW),
        })
    return in_maps


def unshard_out(arr: np.ndarray) -> np.ndarray:
    """Per-core out [NB, HS, C*BW] fp16 -> [C, HS, W] f32."""
    o = arr.reshape(NB, HS, C, BW).transpose(2, 1, 0, 3)
    return np.ascontiguousarray(o).reshape(C, HS, W).astype(np.float32)


def assemble(results) -> np.ndarray:
    out = np.empty((B, C, H, W), dtype=np.float32)
    for core in range(NCORES):
        b, hh = divmod(core, 2)
        out[b, :, hh * HS:(hh + 1) * HS, :] = unshard_out(results[core]["out"])
    return out


def kernel(data: np.ndarray, kernels: np.ndarray) -> np.ndarray:
    from concourse.bass_utils import run_bass_kernel_spmd

    nc = get_program()
    in_maps = make_shards(data, kernels)
    res = run_bass_kernel_spmd(nc, in_maps, list(range(NCORES)))
    return assemble(res.results)


if __name__ == "__main__":
    get_program()
    print("program built OK")


# revision 26
# speedup vs baseline: 1.1203x; 1.1203x over previous
"""Trainium2 Bass kernel: per-pixel 5x5 kernel application (KPN-style).

    out[b,c,y,x] = sum_{i,j} softmax(kernels[b,:,y,x])[i*5+j]
                   * zpad(data)[b,c,y+i,x+j]          (i,j in 0..4, r=2)

Sharding (8 NeuronCores, pure data parallel, no collectives):
    core = (b, H-half): 4 batches x 2 row-bands of 360 rows.

Band layout: partition p = x-band of 10 columns (128 bands x 10 = 1280).
Rows live in the free dimension, so BOTH the di (row) and dj (col) tap
shifts become free-dim AP offsets -- no shift matrices, no partition
crossing.  The host ships the data pre-expanded over dj (D[band, row,
c, dj, x] = data[c, row, 10*band + x + dj - 2]) so one DVE op per
(di, c) covers all five dj taps with 4-byte-aligned 2x-mode APs.

Per accumulation tile (20/40-row warmup tiles, then 45 rows):
    - ACT: E = exp(logits), in place, one op (fp16).
    - DVE: per (di, c): ONE batched product q = E * D ([5, nr, 10]).
    - PE:  identity-lhsT matmuls accumulate the 75 tap planes and the
      25 exp planes into 4 PSUM banks (start/stop per bank).  The
      stationary operand never changes, so the PE stays warm.
    - epilogue (issued one di-group into the NEXT tile so nothing
      stalls on the PSUM-stop semaphores): R = recip_approx(sumexp)
      (DVE) + fp16 cast; P: PSUM->SBUF fp16 copy on ACT (issued ahead
      of the next exp); out = P * R (DVE 2x); store.

DMA: all loads ride SWDGE (gpsimd) so descriptors spray across all 16
SDMA engines with one contiguous run per partition.  The first kE load
is emitted BEFORE the D chunks so nothing queues ahead of the
exp->product critical path at startup.  Stores ride the HWDGE rings
(engines 0-3, otherwise idle).  GpSimd tensor ops measured 6x slower
than DVE and slow concurrent DVE ops -- not used.

kernel(**inputs) takes the FULL inputs and returns the FULL output.
"""

import numpy as np
from numpy.lib.stride_tricks import sliding_window_view

B, C, H, W, KW = 4, 3, 720, 1280, 5
NCORES = 8
HS = H // 2            # 360 output rows per shard
NB = 128               # x-bands (partitions)
BW = 10                # band width (output columns per partition)
DR = HS + 4            # 364 data rows incl. 2+2 halo rows
TAPS = KW * KW

# accumulation tiles: small warmup tiles shorten the startup pipeline,
# a small final tile shortens the drain
TILES = ([(0, 20), (20, 40)] + [(60 + 45 * k, 45) for k in range(6)]
         + [(330, 30)])
# D chunk rows, chunk i loaded behind tile i's kE load; fine-grained so
# no kE load ever queues behind a multi-MB D transfer on the SWDGE ring
DCHUNKS = [(0, 50), (50, 96), (96, 142), (142, 188),
           (188, 256), (256, 320), (320, 364)]

_CACHE = {}


def _build_program():
    import concourse.bacc as bacc
    import concourse.mybir as mybir
    from concourse.bass import AP
    from concourse import tile

    f32 = mybir.dt.float32
    f16 = mybir.dt.float16

    nc = bacc.Bacc(
        "TRN2",
        target_bir_lowering=False,
        debug=False,
        enable_asserts=False,
        num_devices=NCORES,
    )
    d_ke = nc.dram_tensor("ke", [NB, HS, TAPS * BW], f16, kind="ExternalInput")
    d_dx = nc.dram_tensor("dx", [NB, DR, C * KW * BW], f16, kind="ExternalInput")
    d_out = nc.dram_tensor("out", [NB, HS, C * BW], f16, kind="ExternalOutput")

    d_id = nc.inline_tensor(np.eye(NB, dtype=np.float16), "ident")

    KE_ROW = TAPS * BW          # 250 elems per (band, row)
    D_ROW = C * KW * BW         # 150 elems per (band, row)

    with tile.TileContext(nc) as tc:
        with tc.tile_pool(name="const", bufs=1) as cpool, \
             tc.tile_pool(name="kt", bufs=2) as kpool, \
             tc.tile_pool(name="dt", bufs=1) as dpool, \
             tc.tile_pool(name="qt", bufs=6) as qpool, \
             tc.tile_pool(name="rt", bufs=2) as rpool, \
             tc.tile_pool(name="ot", bufs=2) as opool, \
             tc.tile_pool(name="ps", bufs=2, space="PSUM") as ppool:

            id_sb = cpool.tile([NB, NB], f16)
            nc.sync.dma_start(out=id_sb[:], in_=d_id.ap())

            DX = dpool.tile([NB, DR, C, KW, BW], f16, tag="dx")

            st_eng = [nc.sync, nc.scalar]

            # deferred epilogue: normalize+store of tile t runs while
            # tile t+1's products stream, so the DVE never stalls
            # waiting for the PE to drain the accumulation group
            pending = []

            def ep_act():
                # P: PSUM -> SBUF fp16 on ACT.  Issued BEFORE the next
                # exp so it isn't stuck behind a 9.7us ACT op when the
                # DVE-side normalize needs it.
                if not pending:
                    return
                ent = pending[0]
                nr, pacc = ent[2], ent[3]
                Pst = opool.tile([NB, nr, C, BW], f16, tag="pst")
                p_view = AP(
                    pacc[:].tensor, 0,
                    [[4 * 512, NB], [BW, nr], [512, C], [1, BW]],
                )
                nc.scalar.activation(
                    Pst[:], p_view, mybir.ActivationFunctionType.Copy,
                )
                ent[4] = Pst

            def ep_dve():
                # normalize+store, issued one di-group into the next
                # tile so the PSUM-stop semaphores are already clear
                if not pending:
                    return
                t, r0, nr, pacc, Pst = pending.pop(0)
                fd = nr * BW
                R32 = rpool.tile([NB, nr, BW], f32, tag="r32")
                nc.vector.reciprocal_approx_fast(
                    out=R32[:].rearrange("p r x -> p (r x)"),
                    in_=pacc[:, 3, 0:fd],
                )
                R16 = rpool.tile([NB, nr, BW], f16, tag="r16")
                nc.vector.tensor_copy(
                    R16[:].rearrange("p r x -> p (r x)"),
                    R32[:].rearrange("p r x -> p (r x)"),
                )
                outst = opool.tile([NB, nr, C, BW], f16, tag="o")
                r_bc = AP(
                    R16[:].tensor, 0,
                    [[nr * BW, NB], [BW, nr], [0, C], [1, BW]],
                )
                nc.vector.tensor_tensor(
                    outst[:], Pst[:], r_bc, mybir.AluOpType.mult)
                st_eng[t % 2].dma_start(
                    out=d_out.ap()[:, r0:r0 + nr],
                    in_=outst[:].rearrange("p r c x -> p r (c x)"),
                )

            for t, (r0, nr) in enumerate(TILES):
                fd = nr * BW
                kE = kpool.tile([NB, nr, TAPS, BW], f16, tag="ke")
                nc.gpsimd.dma_start(
                    out=kE[:].rearrange("p r t x -> p r (t x)"),
                    in_=d_ke.ap()[:, r0:r0 + nr],
                )
                # D chunks ride the same SWDGE queue, behind the kE
                # loads whose consumers they race
                if t < len(DCHUNKS):
                    ra, rb = DCHUNKS[t]
                    nc.gpsimd.dma_start(
                        out=DX[:, ra:rb].rearrange("p r c k x -> p r (c k x)"),
                        in_=d_dx.ap()[:, ra:rb],
                    )

                ep_act()
                # exp in place: kE holds E from here on
                keflat = kE[:].rearrange("p r t x -> p (r t x)")
                nc.scalar.activation(
                    keflat, keflat, mybir.ActivationFunctionType.Exp,
                )
                eap = kE[:]

                pacc = ppool.tile([NB, 4, 512], f32, tag="pacc")

                for di in range(KW):
                    # products: q = E * D, all 5 dj in one op
                    qs = []
                    for c in range(C):
                        q = qpool.tile([NB, KW, nr, BW], f16, tag="q")
                        e_v = AP(
                            eap.tensor, (KW * di) * BW,
                            [[nr * KE_ROW, NB], [BW, KW], [KE_ROW, nr], [1, BW]],
                        )
                        d_v = AP(
                            DX[:].tensor, (r0 + di) * D_ROW + c * KW * BW,
                            [[DR * D_ROW, NB], [BW, KW], [D_ROW, nr], [1, BW]],
                        )
                        nc.vector.tensor_tensor(
                            q[:], e_v, d_v, mybir.AluOpType.mult)
                        qs.append(q)
                    # sumexp: 5 identity matmuls straight off E (no DVE dep)
                    for k in range(KW):
                        tp = KW * di + k
                        nc.tensor.matmul(
                            out=pacc[:, 3, 0:fd],
                            lhsT=id_sb[:],
                            rhs=eap[:, :, tp, :],
                            start=(tp == 0),
                            stop=(tp == TAPS - 1),
                        )
                    # tap accumulation
                    for c in range(C):
                        for k in range(KW):
                            nc.tensor.matmul(
                                out=pacc[:, c, 0:fd],
                                lhsT=id_sb[:],
                                rhs=qs[c][:, k],
                                start=(di == 0 and k == 0),
                                stop=(di == KW - 1 and k == KW - 1),
                            )
                    if di == 0:
                        ep_dve()

                pending.append([t, r0, nr, pacc, None])

            ep_act()
            ep_dve()

    nc.compile()
    return nc


def get_program():
    if "nc" not in _CACHE:
        _CACHE["nc"] = _build_program()
    return _CACHE["nc"]


def make_shards(data: np.ndarray, kernels: np.ndarray):
    """Full inputs -> per-core input maps (band layout, fp16)."""
    data = np.asarray(data, dtype=np.float32)
    kernels = np.asarray(kernels, dtype=np.float32)

    kf = kernels.astype(np.float16)                   # [B, 25, H, W]
    dpad = np.zeros((B, C, H + 4, W + 6), dtype=np.float16)
    dpad[:, :, 2:H + 2, 2:W + 2] = data

    in_maps = []
    for core in range(NCORES):
        b, hh = divmod(core, 2)
        r0 = hh * HS
        ks = kf[b, :, r0:r0 + HS, :]                  # [25, 360, 1280]
        ke = np.ascontiguousarray(
            ks.reshape(TAPS, HS, NB, BW).transpose(2, 1, 0, 3)
        ).reshape(NB, HS, TAPS * BW)
        dsl = dpad[b, :, r0:r0 + DR, :]               # [3, 364, 1286]
        win = sliding_window_view(dsl, BW, axis=2)    # [3, 364, 1277, 10]
        # dx[p, row, c, dj, x] = data[c, row, 10p + x + dj - 2]
        dx = np.empty((NB, DR, C, KW, BW), dtype=np.float16)
        for dj in range(KW):
            dx[:, :, :, dj, :] = (
                win[:, :, dj:dj + NB * BW:BW].transpose(2, 1, 0, 3))
        in_maps.append({
            "ke": ke,
            "dx": dx.reshape(NB, DR, C * KW * BW),
        })
    return in_maps


def unshard_out(arr: np.ndarray) -> np.ndarray:
    """Per-core out [NB, HS, C*BW] fp16 -> [C, HS, W] f32."""
    o = arr.reshape(NB, HS, C, BW).transpose(2, 1, 0, 3)
    return np.ascontiguousarray(o).reshape(C, HS, W).astype(np.float32)


def assemble(results) -> np.ndarray:
    out = np.empty((B, C, H, W), dtype=np.float32)
    for core in range(NCORES):
        b, hh = divmod(core, 2)
        out[b, :, hh * HS:(hh + 1) * HS, :] = unshard_out(results[core]["out"])
    return out


def kernel(data: np.ndarray, kernels: np.ndarray) -> np.ndarray:
    from concourse.bass_utils import run_bass_kernel_spmd

    nc = get_program()
    in_maps = make_shards(data, kernels)
    res = run_bass_kernel_spmd(nc, in_maps, list(range(NCORES)))
    return assemble(res.results)


if __name__ == "__main__":
    get_program()
    print("program built OK")


# revision 27
# speedup vs baseline: 1.1703x; 1.0446x over previous
"""Trainium2 Bass kernel: per-pixel 5x5 kernel application (KPN-style).

    out[b,c,y,x] = sum_{i,j} softmax(kernels[b,:,y,x])[i*5+j]
                   * zpad(data)[b,c,y+i,x+j]          (i,j in 0..4, r=2)

Sharding (8 NeuronCores, pure data parallel, no collectives):
    core = (b, H-half): 4 batches x 2 row-bands of 360 rows.

Band layout: partition p = x-band of 10 columns (128 bands x 10 = 1280).
Rows live in the free dimension, so BOTH the di (row) and dj (col) tap
shifts become free-dim AP offsets -- no shift matrices, no partition
crossing.  The host ships the data pre-expanded over dj (D[band, row,
c, dj, x] = data[c, row, 10*band + x + dj - 2]) so one DVE op per
(di, c) covers all five dj taps with 4-byte-aligned 2x-mode APs.

Per accumulation tile (20/40-row warmup tiles, then 45 rows):
    - ACT: E = exp(logits), in place, one op (fp16).
    - DVE: per (di, c): ONE batched product q = E * D ([5, nr, 10]).
    - PE:  identity-lhsT matmuls accumulate the 75 tap planes and the
      25 exp planes into 4 PSUM banks (start/stop per bank).  The
      stationary operand never changes, so the PE stays warm.
    - epilogue (issued one di-group into the NEXT tile so nothing
      stalls on the PSUM-stop semaphores): R = recip_approx(sumexp)
      (DVE) + fp16 cast; P: PSUM->SBUF fp16 copy on ACT (issued ahead
      of the next exp); out = P * R (DVE 2x); store.

DMA: all loads ride SWDGE (gpsimd) so descriptors spray across all 16
SDMA engines with one contiguous run per partition.  The first kE load
is emitted BEFORE the D chunks so nothing queues ahead of the
exp->product critical path at startup.  Stores ride the HWDGE rings
(engines 0-3, otherwise idle).  GpSimd tensor ops measured 6x slower
than DVE and slow concurrent DVE ops -- not used.

kernel(**inputs) takes the FULL inputs and returns the FULL output.
"""

import numpy as np
from numpy.lib.stride_tricks import sliding_window_view

B, C, H, W, KW = 4, 3, 720, 1280, 5
NCORES = 8
HS = H // 2            # 360 output rows per shard
NB = 128               # x-bands (partitions)
BW = 10                # band width (output columns per partition)
DR = HS + 4            # 364 data rows incl. 2+2 halo rows
TAPS = KW * KW

# accumulation tiles: small warmup tiles shorten the startup pipeline,
# a small final tile shortens the drain
TILES = ([(0, 20), (20, 40)] + [(60 + 45 * k, 45) for k in range(6)]
         + [(330, 30)])
# D chunk rows, chunk i loaded behind tile i's kE load; fine-grained so
# no kE load ever queues behind a multi-MB D transfer on the SWDGE ring
DCHUNKS = [(0, 50), (50, 96), (96, 142), (142, 188),
           (188, 256), (256, 320), (320, 364)]

_CACHE = {}


def _build_program():
    import concourse.bacc as bacc
    import concourse.mybir as mybir
    from concourse.bass import AP
    from concourse import tile

    f32 = mybir.dt.float32
    f16 = mybir.dt.float16

    nc = bacc.Bacc(
        "TRN2",
        target_bir_lowering=False,
        debug=False,
        enable_asserts=False,
        num_devices=NCORES,
    )
    d_ke = nc.dram_tensor("ke", [NB, HS, TAPS * BW], f16, kind="ExternalInput")
    d_dx = nc.dram_tensor("dx", [NB, DR, C * KW * BW], f16, kind="ExternalInput")
    d_out = nc.dram_tensor("out", [NB, HS, C * BW], f16, kind="ExternalOutput")

    d_id = nc.inline_tensor(np.eye(NB, dtype=np.float16), "ident")

    KE_ROW = TAPS * BW          # 250 elems per (band, row)
    D_ROW = C * KW * BW         # 150 elems per (band, row)

    with tile.TileContext(nc) as tc:
        with tc.tile_pool(name="const", bufs=1) as cpool, \
             tc.tile_pool(name="kt", bufs=3) as kpool, \
             tc.tile_pool(name="dt", bufs=1) as dpool, \
             tc.tile_pool(name="qt", bufs=4) as qpool, \
             tc.tile_pool(name="rt", bufs=2) as rpool, \
             tc.tile_pool(name="ot", bufs=2) as opool, \
             tc.tile_pool(name="ps", bufs=2, space="PSUM") as ppool:

            id_sb = cpool.tile([NB, NB], f16)
            nc.sync.dma_start(out=id_sb[:], in_=d_id.ap())

            DX = dpool.tile([NB, DR, C, KW, BW], f16, tag="dx")

            st_eng = [nc.sync, nc.scalar]

            # deferred epilogue: normalize+store of tile t runs while
            # tile t+1's products stream, so the DVE never stalls
            # waiting for the PE to drain the accumulation group
            pending = []

            def ep_act():
                # P: PSUM -> SBUF fp16 on ACT.  Issued BEFORE the next
                # exp so it isn't stuck behind a 9.7us ACT op when the
                # DVE-side normalize needs it.
                if not pending:
                    return
                ent = pending[0]
                nr, pacc = ent[2], ent[3]
                Pst = opool.tile([NB, nr, C, BW], f16, tag="pst", bufs=1)
                p_view = AP(
                    pacc[:].tensor, 0,
                    [[4 * 512, NB], [BW, nr], [512, C], [1, BW]],
                )
                nc.scalar.activation(
                    Pst[:], p_view, mybir.ActivationFunctionType.Copy,
                )
                ent[4] = Pst

            def ep_dve():
                # normalize+store, issued one di-group into the next
                # tile so the PSUM-stop semaphores are already clear
                if not pending:
                    return
                t, r0, nr, pacc, Pst = pending.pop(0)
                fd = nr * BW
                R32 = rpool.tile([NB, nr, BW], f32, tag="r32", bufs=1)
                nc.vector.reciprocal_approx_fast(
                    out=R32[:].rearrange("p r x -> p (r x)"),
                    in_=pacc[:, 3, 0:fd],
                )
                R16 = rpool.tile([NB, nr, BW], f16, tag="r16", bufs=1)
                nc.vector.tensor_copy(
                    R16[:].rearrange("p r x -> p (r x)"),
                    R32[:].rearrange("p r x -> p (r x)"),
                )
                outst = opool.tile([NB, nr, C, BW], f16, tag="o")
                r_bc = AP(
                    R16[:].tensor, 0,
                    [[nr * BW, NB], [BW, nr], [0, C], [1, BW]],
                )
                nc.vector.tensor_tensor(
                    outst[:], Pst[:], r_bc, mybir.AluOpType.mult)
                st_eng[t % 2].dma_start(
                    out=d_out.ap()[:, r0:r0 + nr],
                    in_=outst[:].rearrange("p r c x -> p r (c x)"),
                )

            for t, (r0, nr) in enumerate(TILES):
                fd = nr * BW
                kE = kpool.tile([NB, nr, TAPS, BW], f16, tag="ke")
                nc.gpsimd.dma_start(
                    out=kE[:].rearrange("p r t x -> p r (t x)"),
                    in_=d_ke.ap()[:, r0:r0 + nr],
                )
                # D chunks ride the same SWDGE queue, behind the kE
                # loads whose consumers they race
                if t < len(DCHUNKS):
                    ra, rb = DCHUNKS[t]
                    nc.gpsimd.dma_start(
                        out=DX[:, ra:rb].rearrange("p r c k x -> p r (c k x)"),
                        in_=d_dx.ap()[:, ra:rb],
                    )

                ep_act()
                # exp in place: kE holds E from here on
                keflat = kE[:].rearrange("p r t x -> p (r t x)")
                nc.scalar.activation(
                    keflat, keflat, mybir.ActivationFunctionType.Exp,
                )
                eap = kE[:]

                pacc = ppool.tile([NB, 4, 512], f32, tag="pacc")

                for di in range(KW):
                    # products: q = E * D, all 5 dj in one op
                    qs = []
                    for c in range(C):
                        q = qpool.tile([NB, KW, nr, BW], f16, tag="q")
                        e_v = AP(
                            eap.tensor, (KW * di) * BW,
                            [[nr * KE_ROW, NB], [BW, KW], [KE_ROW, nr], [1, BW]],
                        )
                        d_v = AP(
                            DX[:].tensor, (r0 + di) * D_ROW + c * KW * BW,
                            [[DR * D_ROW, NB], [BW, KW], [D_ROW, nr], [1, BW]],
                        )
                        nc.vector.tensor_tensor(
                            q[:], e_v, d_v, mybir.AluOpType.mult)
                        qs.append(q)
                    # sumexp: 5 identity matmuls straight off E (no DVE dep)
                    for k in range(KW):
                        tp = KW * di + k
                        nc.tensor.matmul(
                            out=pacc[:, 3, 0:fd],
                            lhsT=id_sb[:],
                            rhs=eap[:, :, tp, :],
                            start=(tp == 0),
                            stop=(tp == TAPS - 1),
                        )
                    # tap accumulation
                    for c in range(C):
                        for k in range(KW):
                            nc.tensor.matmul(
                                out=pacc[:, c, 0:fd],
                                lhsT=id_sb[:],
                                rhs=qs[c][:, k],
                                start=(di == 0 and k == 0),
                                stop=(di == KW - 1 and k == KW - 1),
                            )
                    if di == 0:
                        ep_dve()

                pending.append([t, r0, nr, pacc, None])

            ep_act()
            ep_dve()

    nc.compile()
    return nc


def get_program():
    if "nc" not in _CACHE:
        _CACHE["nc"] = _build_program()
    return _CACHE["nc"]


def make_shards(data: np.ndarray, kernels: np.ndarray):
    """Full inputs -> per-core input maps (band layout, fp16)."""
    data = np.asarray(data, dtype=np.float32)
    kernels = np.asarray(kernels, dtype=np.float32)

    kf = kernels.astype(np.float16)                   # [B, 25, H, W]
    dpad = np.zeros((B, C, H + 4, W + 6), dtype=np.float16)
    dpad[:, :, 2:H + 2, 2:W + 2] = data

    in_maps = []
    for core in range(NCORES):
        b, hh = divmod(core, 2)
        r0 = hh * HS
        ks = kf[b, :, r0:r0 + HS, :]                  # [25, 360, 1280]
        ke = np.ascontiguousarray(
            ks.reshape(TAPS, HS, NB, BW).transpose(2, 1, 0, 3)
        ).reshape(NB, HS, TAPS * BW)
        dsl = dpad[b, :, r0:r0 + DR, :]               # [3, 364, 1286]
        win = sliding_window_view(dsl, BW, axis=2)    # [3, 364, 1277, 10]
        # dx[p, row, c, dj, x] = data[c, row, 10p + x + dj - 2]
        dx = np.empty((NB, DR, C, KW, BW), dtype=np.float16)
        for dj in range(KW):
            dx[:, :, :, dj, :] = (
                win[:, :, dj:dj + NB * BW:BW].transpose(2, 1, 0, 3))
        in_maps.append({
            "ke": ke,
            "dx": dx.reshape(NB, DR, C * KW * BW),
        })
    return in_maps


def unshard_out(arr: np.ndarray) -> np.ndarray:
    """Per-core out [NB, HS, C*BW] fp16 -> [C, HS, W] f32."""
    o = arr.reshape(NB, HS, C, BW).transpose(2, 1, 0, 3)
    return np.ascontiguousarray(o).reshape(C, HS, W).astype(np.float32)


def assemble(results) -> np.ndarray:
    out = np.empty((B, C, H, W), dtype=np.float32)
    for core in range(NCORES):
        b, hh = divmod(core, 2)
        out[b, :, hh * HS:(hh + 1) * HS, :] = unshard_out(results[core]["out"])
    return out


def kernel(data: np.ndarray, kernels: np.ndarray) -> np.ndarray:
    from concourse.bass_utils import run_bass_kernel_spmd

    nc = get_program()
    in_maps = make_shards(data, kernels)
    res = run_bass_kernel_spmd(nc, in_maps, list(range(NCORES)))
    return assemble(res.results)


if __name__ == "__main__":
    get_program()
    print("program built OK")
